# revision 7
# baseline (speedup 1.0000x reference)
"""Trainium2 Bass kernel for nn_AttentionSortNet (sparse_attention).

Per bh slice (data-parallel over bh across 8 cores):
  b_q = bucket-mean(q), b_k = bucket-mean(k)          (64 buckets x 128 elems)
  sq = b_q + q_pos, sk = b_k + k_pos
  R  = sq @ sk^T                                       (64 x 64)
  K  = exp((ln(relu(R)+eps) + gumbel) / T)
  8x Sinkhorn row/col normalization; out = final E

Device mapping (per core, 4 bh = 2 bh-pairs):
  - q/k pair loads: 1 MiB SWDGE DMAs per (pair, tensor) into tiles
    [128, 2048] with partition = (bh-in-pair, bucket), free = (seq r, dim d).
    The gpsimd queue carries ONLY these chunk DMAs. The last stream
    (pair-1 k) uses 8 x 512 KiB chunks to shrink the post-DMA tail.
  - bucket SUMS (not means): q via accumulating fp32r matmuls on the PE
    (identity stationary); k via DVE strided reduces (PE and DVE each
    handle ~half the 16 MiB so both fit inside the DMA window).
    The /128 mean and /128^2 einsum scales are folded host-side into
    pos*128 and gumbel - ln(128^2), which is exact.
  - ACT tables: a custom insert_act_table_loads pins Ln and Exp to the
    combined natural_log_exp_and_others table, eliminating the 1.28us
    table reload that otherwise sits between Ln and Exp per pair.
  - Sinkhorn via scaling vectors instead of 16 full-matrix transposes:
      a_t = 1/(K b_{t-1}),  b_t = 1/(K^T a_t),  b_0 = 1
    K per pair is packed BLOCK-DIAGONALLY in a [128,128] tile (bh even in
    [0:64,0:64], bh odd in [64:128,64:128], zeros elsewhere) so one
    1-column bf16 matvec serves both bh. a_1 comes free from the Exp
    activation's accum_out row sums. Final E = diag(a_8) K diag(b_8) is
    assembled as diag(a) * transpose(diag(b) K^T) with fp32 K.
  - Pair 0's whole chain overlaps the DMA phase; only pair 1's chain
    trails the last chunk.
"""
import sys

sys.path.insert(0, "/opt/trn_rl_repo")

import numpy as np

import bass_rust as _bass_rust
import concourse.bass as bass
import concourse.bacc as bacc
import concourse.mybir as mybir
from concourse import tile
from concourse.bass_utils import run_bass_kernel_spmd
from concourse.hw_specs import get_activation_tables

HEADS = 8
BUCKETS = 64
DIM = 64
TEMP = 0.7
EPS = 1e-6
N_CORES = 8
BH = 32
SEQ = 8192
NBH = BH // N_CORES        # 4 bh per core
PAIRS = NBH // 2           # 2 bh-pairs per core
SINKHORN_ITER = 8
SCALE = 128.0 * 128.0      # bucket-sum (not mean) einsum scale, folded on host
EPS_S = EPS * SCALE        # matching eps for ln(relu(R_scaled) + eps_s)

F32 = mybir.dt.float32
F32R = mybir.dt.float32r
BF16 = mybir.dt.bfloat16
AF = mybir.ActivationFunctionType
AX = mybir.AxisListType
ALU = mybir.AluOpType

COMBINED_TABLE = "natural_log_exp_and_others"


class _Bacc(bacc.Bacc):
    """Bacc whose ACT-table placement serves Ln and Exp from the single
    combined table, so alternating Ln/Exp never reloads tables."""

    def insert_act_table_loads(self):
        has_activation = any(
            isinstance(i, mybir.InstActivation)
            for b in self.main_func.blocks
            for i in b.instructions
        )
        if not has_activation:
            return
        tables = []
        for name, fns in get_activation_tables(self.m.arch).items():
            if name != COMBINED_TABLE:
                fns = fns - {AF.Ln, AF.Exp}
            tables.append((name, fns))
        _bass_rust.insert_act_table_loads(self, tables)


def _build_program():
    nc = _Bacc("TRN2", target_bir_lowering=False, debug=False, num_devices=N_CORES)

    q_d = nc.dram_tensor("q", [NBH, SEQ, DIM], F32, kind="ExternalInput")
    k_d = nc.dram_tensor("k", [NBH, SEQ, DIM], F32, kind="ExternalInput")
    qp_d = nc.dram_tensor("qpos", [NBH, BUCKETS, DIM], F32, kind="ExternalInput")
    kp_d = nc.dram_tensor("kpos", [NBH, BUCKETS, DIM], F32, kind="ExternalInput")
    g_d = nc.dram_tensor("gumbel", [NBH, BUCKETS, BUCKETS], F32, kind="ExternalInput")
    id_d = nc.dram_tensor("ident", [128, 128], F32, kind="ExternalInput")
    idr_d = nc.dram_tensor("identr", [128, 128], F32R, kind="ExternalInput")
    out_d = nc.dram_tensor("out", [NBH, BUCKETS, BUCKETS], F32, kind="ExternalOutput")

    with tile.TileContext(nc) as tc:
        with (
            tc.tile_pool(name="const", bufs=1) as constp,
            tc.tile_pool(name="data", bufs=6) as datap,
            tc.tile_pool(name="work", bufs=2) as workp,
            tc.tile_pool(name="persist", bufs=1) as persistp,
            tc.tile_pool(name="chain", bufs=2) as chainp,
            tc.tile_pool(name="pacc", bufs=2, space=bass.MemorySpace.PSUM) as pacc,
            tc.tile_pool(name="ptr", bufs=1, space=bass.MemorySpace.PSUM) as ptr,
            tc.tile_pool(name="pR", bufs=1, space=bass.MemorySpace.PSUM) as pR,
            tc.tile_pool(name="pT", bufs=2, space=bass.MemorySpace.PSUM) as pT,
            tc.tile_pool(name="pmv", bufs=2, space=bass.MemorySpace.PSUM) as pmv,
        ):
            # identities via HWDGE: fp32 for transposes, fp32r (pre-rounded
            # on host; 0/1 are exact) as the q bucket-sum stationary.
            identr = constp.tile([128, 128], F32R, tag="identr")
            nc.sync.dma_start(identr[:], idr_d[:])
            ident = constp.tile([128, 128], F32, tag="ident")
            nc.sync.dma_start(ident[:], id_d[:])

            epst = constp.tile([128, 1], F32, tag="eps")
            nc.vector.memset(epst[:], EPS_S)

            # ACT warm-up: loads the combined Ln+Exp table once.
            tw = constp.tile([128, 1], F32, tag="tw")
            nc.scalar.activation(tw[:], epst[:], AF.Ln, bias=epst[:])
            nc.scalar.activation(tw[:], tw[:], AF.Exp)

            # pos embeddings / gumbel, pair-stacked: [128, 2, 64] with
            # partitions 0:64 = bh {0, 2} (even in pair), 64:128 = bh {1, 3}.
            def load_stacked(dst, src_handle):
                v = src_handle[:].rearrange("(p v) r d -> v r p d", p=2, v=2)
                nc.sync.dma_start(dst[0:64, :, :], v[0])
                nc.sync.dma_start(dst[64:128, :, :], v[1])

            posq = persistp.tile([128, PAIRS, DIM], F32, tag="posq")
            load_stacked(posq, qp_d)
            posk = persistp.tile([128, PAIRS, DIM], F32, tag="posk")
            load_stacked(posk, kp_d)
            gum = persistp.tile([128, PAIRS, BUCKETS], F32, tag="gum")
            load_stacked(gum, g_d)

            # block-diagonal K tiles, zeroed once so off-diagonal quadrants
            # stay 0 for the packed matvecs
            Aps = []
            for pi in range(PAIRS):
                Ap = persistp.tile([128, 128], F32, tag=f"Ap{pi}")
                nc.vector.memset(Ap[:], 0.0)
                Aps.append(Ap)

            seed = persistp.tile([128, PAIRS], F32, tag="seed")  # K row sums

            for pi in range(PAIRS):
                # ---- q bucket sums on the PE (psum-accumulating fp32r) ----
                qview = q_d[2 * pi : 2 * pi + 2].rearrange(
                    "b (bu c rl) d -> (b bu) c (rl d)", bu=BUCKETS, c=4, rl=32
                )
                acc = pacc.tile([128, DIM, 8], F32, tag="acc")
                for c in range(4):
                    chunk = datap.tile([128, 32 * DIM], F32R, tag="dq")
                    nc.gpsimd.dma_start(chunk[:], qview[:, c])
                    dv = chunk[:].rearrange(
                        "p (ro ri d) -> p ro d ri", ro=4, ri=8, d=DIM
                    )
                    for j in range(4):
                        nc.tensor.matmul(
                            acc[:],
                            identr[:],
                            dv[:, j],
                            start=(c == 0 and j == 0),
                            stop=(c == 3 and j == 3),
                        )
                s_sb = workp.tile([128, DIM], F32, tag="s")
                nc.vector.reduce_sum(s_sb[:], acc[:], axis=AX.X)
                nc.vector.tensor_add(s_sb[:], s_sb[:], posq[:, pi, :])
                tps = ptr.tile([64, 128], F32, tag="tp")
                nc.tensor.transpose(tps[:], s_sb[:], ident[:])
                sTq = persistp.tile([64, 128], F32, tag=f"sTq{pi}")
                nc.vector.tensor_copy(sTq[:], tps[:])

                # ---- k bucket sums on the DVE (strided chunk reduces) ----
                nchunks = 8 if pi == PAIRS - 1 else 4
                rl = 128 // nchunks
                kview = k_d[2 * pi : 2 * pi + 2].rearrange(
                    "b (bu c rl) d -> (b bu) c (rl d)", bu=BUCKETS, c=nchunks, rl=rl
                )
                ks = workp.tile([128, DIM], F32, tag=f"ks{pi}", bufs=1)
                nc.vector.tensor_copy(ks[:], posk[:, pi, :])
                for c in range(nchunks):
                    kchunk = datap.tile(
                        [128, rl * DIM], F32, tag=f"dk{nchunks}", bufs=nchunks
                    )
                    nc.gpsimd.dma_start(kchunk[:], kview[:, c])
                    kred = workp.tile([128, DIM], F32, tag="kred")
                    nc.vector.reduce_sum(
                        kred[:],
                        kchunk[:].rearrange("p (rr d) -> p d rr", rr=rl, d=DIM),
                        axis=AX.X,
                    )
                    nc.vector.tensor_add(ks[:], ks[:], kred[:])
                tpk = ptr.tile([64, 128], F32, tag="tp")
                nc.tensor.transpose(tpk[:], ks[:], ident[:])
                sTk = persistp.tile([64, 128], F32, tag=f"sTk{pi}")
                nc.vector.tensor_copy(sTk[:], tpk[:])

                # R[i, j] = sum_d sq[i, d] sk[j, d]; bh pair stacked on partitions
                Rps = pR.tile([128, BUCKETS], F32, tag="R")
                for v in range(2):
                    nc.tensor.matmul(
                        Rps[64 * v : 64 * (v + 1), :],
                        sTq[:, 64 * v : 64 * (v + 1)],
                        sTk[:, 64 * v : 64 * (v + 1)],
                        start=True,
                        stop=True,
                    )

                # K = exp((ln(relu(R)+eps) + g) / T) written block-diagonally,
                # with row sums accumulated as the first Sinkhorn seed
                # (b_0 = ones => a_1 = 1/rowsums).
                t1 = workp.tile([128, BUCKETS], F32, tag="t1a")
                nc.vector.tensor_scalar_max(t1[:], Rps[:], 0.0)
                t2 = workp.tile([128, BUCKETS], F32, tag="t1b")
                nc.scalar.activation(t2[:], t1[:], AF.Ln, bias=epst[:])
                nc.vector.tensor_add(t2[:], t2[:], gum[:, pi, :])
                Ap = Aps[pi]
                nc.scalar.activation(
                    Ap[0:64, 0:64], t2[0:64, :], AF.Exp,
                    scale=1.0 / TEMP,
                    accum_out=seed[0:64, pi : pi + 1],
                )
                nc.scalar.activation(
                    Ap[64:128, 64:128], t2[64:128, :], AF.Exp,
                    scale=1.0 / TEMP,
                    accum_out=seed[64:128, pi : pi + 1],
                )

                # bf16 copies of K and K^T for the matvec chain; fp32 K^T for
                # the final assembly.
                with nc.allow_low_precision("sinkhorn matvecs in bf16"):
                    Kbf = persistp.tile([128, 128], BF16, tag=f"Kbf{pi}")
                    nc.vector.tensor_copy(Kbf[:], Ap[:])
                    a_bf = chainp.tile([128, 1], BF16, tag=f"a{pi}")
                    nc.vector.reciprocal(a_bf[:], seed[:, pi : pi + 1])
                    tpA = pT.tile([128, 128], F32, tag="tpA")
                    nc.tensor.transpose(tpA[:], Ap[:], ident[:])
                    ApT = persistp.tile([128, 128], F32, tag=f"ApT{pi}")
                    nc.vector.tensor_copy(ApT[:], tpA[:])
                    KTbf = persistp.tile([128, 128], BF16, tag=f"KTbf{pi}")
                    nc.vector.tensor_copy(KTbf[:], tpA[:])

                    # Sinkhorn chain on scaling vectors:
                    #   a_t = 1/(K b_{t-1}) [seed for t=1], b_t = 1/(K^T a_t)
                    a_f32 = persistp.tile([128, 1], F32, tag=f"af{pi}")
                    b_f32 = persistp.tile([128, 1], F32, tag=f"bf{pi}")
                    for t in range(1, SINKHORN_ITER + 1):
                        # b_t = 1/(K^T a_t): contraction over i -> lhsT = K
                        mv = pmv.tile([128, 1], F32, tag="mv")
                        nc.tensor.matmul(mv[:], Kbf[:], a_bf[:], start=True, stop=True)
                        if t == SINKHORN_ITER:
                            nc.vector.reciprocal(b_f32[:], mv[:])
                        else:
                            b_bf = chainp.tile([128, 1], BF16, tag=f"b{pi}")
                            nc.vector.reciprocal(b_bf[:], mv[:])
                            # a_{t+1} = 1/(K b_t): contraction over j -> lhsT = K^T
                            mv2 = pmv.tile([128, 1], F32, tag="mv")
                            nc.tensor.matmul(mv2[:], KTbf[:], b_bf[:], start=True, stop=True)
                            a_bf = chainp.tile([128, 1], BF16, tag=f"a{pi}")
                            nc.vector.reciprocal(a_bf[:], mv2[:])
                            if t == SINKHORN_ITER - 1:
                                nc.vector.reciprocal(a_f32[:], mv2[:])

                # final E = diag(a_8) K diag(b_8) = diag(a) (diag(b) K^T)^T
                Tb = workp.tile([128, 128], F32, tag="Tb")
                nc.vector.tensor_scalar_mul(Tb[:], ApT[:], b_f32[:])
                tpF = pT.tile([128, 128], F32, tag="tpA")
                nc.tensor.transpose(tpF[:], Tb[:], ident[:])
                osb = persistp.tile([128, 128], F32, tag=f"osb{pi}")
                nc.vector.tensor_scalar_mul(osb[:], tpF[:], a_f32[:])

                # split the two quadrant DMAs across engines so descriptor
                # generation for the tail pair is parallel
                nc.sync.dma_start(out_d[2 * pi], osb[0:64, 0:64])
                nc.scalar.dma_start(out_d[2 * pi + 1], osb[64:128, 64:128])

    nc.compile()
    return nc


_NC = None


def _get_program():
    global _NC
    if _NC is None:
        _NC = _build_program()
    return _NC


def _make_in_maps(inputs):
    q = np.ascontiguousarray(inputs["q"], dtype=np.float32)
    k = np.ascontiguousarray(inputs["k"], dtype=np.float32)
    qpe = np.asarray(inputs["q_pos_emb"], dtype=np.float32)
    kpe = np.asarray(inputs["k_pos_emb"], dtype=np.float32)
    g = np.ascontiguousarray(inputs["gumbel"], dtype=np.float32)

    b = BH // HEADS
    # device computes bucket SUMS: fold the /128 mean into pos*128 and the
    # resulting /128^2 einsum scale into gumbel - ln(128^2) (exact in the
    # log domain of the sinkhorn kernel)
    qpos = (np.broadcast_to(qpe, (b, HEADS, BUCKETS, DIM)).reshape(BH, BUCKETS, DIM)
            * 128.0).astype(np.float32)
    kpos = (np.broadcast_to(kpe, (b, HEADS, BUCKETS, DIM)).reshape(BH, BUCKETS, DIM)
            * 128.0).astype(np.float32)
    gshift = (g.astype(np.float64) - np.log(SCALE)).astype(np.float32)
    ident = np.eye(128, dtype=np.float32)

    in_maps = []
    for c in range(N_CORES):
        sl = slice(NBH * c, NBH * (c + 1))
        in_maps.append(
            {
                "q": np.ascontiguousarray(q[sl]),
                "k": np.ascontiguousarray(k[sl]),
                "qpos": np.ascontiguousarray(qpos[sl]),
                "kpos": np.ascontiguousarray(kpos[sl]),
                "gumbel": np.ascontiguousarray(gshift[sl]),
                "ident": ident,
                "identr": ident,
            }
        )
    return in_maps


def run(inputs, trace=False):
    nc = _get_program()
    in_maps = _make_in_maps(inputs)
    res = run_bass_kernel_spmd(
        nc, in_maps, core_ids=list(range(N_CORES)), trace=trace
    )
    out = np.concatenate(
        [res.results[c]["out"] for c in range(N_CORES)], axis=0
    ).astype(np.float32)
    return out, res


def kernel(**inputs) -> np.ndarray:
    out, _ = run(inputs, trace=False)
    return out


# revision 8
# speedup vs baseline: 1.0279x; 1.0279x over previous
"""Trainium2 Bass kernel for nn_AttentionSortNet (sparse_attention).

Per bh slice (data-parallel over bh across 8 cores):
  b_q = bucket-mean(q), b_k = bucket-mean(k)          (64 buckets x 128 elems)
  sq = b_q + q_pos, sk = b_k + k_pos
  R  = sq @ sk^T                                       (64 x 64)
  K  = exp((ln(relu(R)+eps) + gumbel) / T)
  8x Sinkhorn row/col normalization; out = final E

Device mapping (per core, 4 bh = 2 bh-pairs):
  - q/k pair loads: 1 MiB SWDGE DMAs per (pair, tensor) into tiles
    [128, 2048] with partition = (bh-in-pair, bucket), free = (seq r, dim d).
    The gpsimd queue carries ONLY these chunk DMAs. The last stream
    (pair-1 k) uses 8 x 512 KiB chunks to shrink the post-DMA tail.
  - bucket SUMS (not means): q via accumulating fp32r matmuls on the PE
    (identity stationary); k via DVE strided reduces (PE and DVE each
    handle ~half the 16 MiB so both fit inside the DMA window).
    The /128 mean and /128^2 einsum scales are folded host-side into
    pos*128 and gumbel - ln(128^2), which is exact.
  - ACT tables: a custom insert_act_table_loads pins Ln and Exp to the
    combined natural_log_exp_and_others table, eliminating the 1.28us
    table reload that otherwise sits between Ln and Exp per pair.
  - Sinkhorn via scaling vectors instead of 16 full-matrix transposes:
      a_t = 1/(K b_{t-1}),  b_t = 1/(K^T a_t),  b_0 = 1
    K per pair is packed BLOCK-DIAGONALLY in a [128,128] tile (bh even in
    [0:64,0:64], bh odd in [64:128,64:128], zeros elsewhere) so one
    1-column bf16 matvec serves both bh. a_1 comes free from the Exp
    activation's accum_out row sums. Final E = diag(a_8) K diag(b_8) is
    assembled as diag(a) * transpose(diag(b) K^T) with fp32 K.
  - Pair 0's whole chain overlaps the DMA phase; only pair 1's chain
    trails the last chunk.
"""
import sys

sys.path.insert(0, "/opt/trn_rl_repo")

import numpy as np

import bass_rust as _bass_rust
import concourse.bass as bass
import concourse.bacc as bacc
import concourse.mybir as mybir
from concourse import tile
from concourse.bass_utils import run_bass_kernel_spmd
from concourse.hw_specs import get_activation_tables

HEADS = 8
BUCKETS = 64
DIM = 64
TEMP = 0.7
EPS = 1e-6
N_CORES = 8
BH = 32
SEQ = 8192
NBH = BH // N_CORES        # 4 bh per core
PAIRS = NBH // 2           # 2 bh-pairs per core
SINKHORN_ITER = 8
SCALE = 128.0 * 128.0      # bucket-sum (not mean) einsum scale, folded on host
EPS_S = EPS * SCALE        # matching eps for ln(relu(R_scaled) + eps_s)

F32 = mybir.dt.float32
F32R = mybir.dt.float32r
BF16 = mybir.dt.bfloat16
AF = mybir.ActivationFunctionType
AX = mybir.AxisListType
ALU = mybir.AluOpType

COMBINED_TABLE = "natural_log_exp_and_others"


class _Bacc(bacc.Bacc):
    """Bacc whose ACT-table placement serves Ln and Exp from the single
    combined table, so alternating Ln/Exp never reloads tables."""

    def insert_act_table_loads(self):
        has_activation = any(
            isinstance(i, mybir.InstActivation)
            for b in self.main_func.blocks
            for i in b.instructions
        )
        if not has_activation:
            return
        tables = []
        for name, fns in get_activation_tables(self.m.arch).items():
            if name != COMBINED_TABLE:
                fns = fns - {AF.Ln, AF.Exp}
            tables.append((name, fns))
        _bass_rust.insert_act_table_loads(self, tables)


def _build_program():
    nc = _Bacc("TRN2", target_bir_lowering=False, debug=False, num_devices=N_CORES)

    q_d = nc.dram_tensor("q", [NBH, SEQ, DIM], F32, kind="ExternalInput")
    k_d = nc.dram_tensor("k", [NBH, SEQ, DIM], F32, kind="ExternalInput")
    qp_d = nc.dram_tensor("qpos", [NBH, BUCKETS, DIM], F32, kind="ExternalInput")
    kp_d = nc.dram_tensor("kpos", [NBH, BUCKETS, DIM], F32, kind="ExternalInput")
    g_d = nc.dram_tensor("gumbel", [NBH, BUCKETS, BUCKETS], F32, kind="ExternalInput")
    id_d = nc.dram_tensor("ident", [128, 128], F32, kind="ExternalInput")
    idr_d = nc.dram_tensor("identr", [128, 128], F32R, kind="ExternalInput")
    out_d = nc.dram_tensor("out", [NBH, BUCKETS, BUCKETS], F32, kind="ExternalOutput")

    with tile.TileContext(nc) as tc:
        with (
            tc.tile_pool(name="const", bufs=1) as constp,
            tc.tile_pool(name="data", bufs=6) as datap,
            tc.tile_pool(name="work", bufs=2) as workp,
            tc.tile_pool(name="persist", bufs=1) as persistp,
            tc.tile_pool(name="chain", bufs=2) as chainp,
            tc.tile_pool(name="pacc", bufs=2, space=bass.MemorySpace.PSUM) as pacc,
            tc.tile_pool(name="ptr", bufs=1, space=bass.MemorySpace.PSUM) as ptr,
            tc.tile_pool(name="pR", bufs=1, space=bass.MemorySpace.PSUM) as pR,
            tc.tile_pool(name="pT", bufs=2, space=bass.MemorySpace.PSUM) as pT,
            tc.tile_pool(name="pmv", bufs=2, space=bass.MemorySpace.PSUM) as pmv,
        ):
            # identities via HWDGE: fp32 for transposes, fp32r (pre-rounded
            # on host; 0/1 are exact) as the q bucket-sum stationary.
            identr = constp.tile([128, 128], F32R, tag="identr")
            nc.sync.dma_start(identr[:], idr_d[:])
            ident = constp.tile([128, 128], F32, tag="ident")
            nc.sync.dma_start(ident[:], id_d[:])

            epst = constp.tile([128, 1], F32, tag="eps")
            nc.vector.memset(epst[:], EPS_S)

            # ACT warm-up: loads the combined Ln+Exp table once.
            tw = constp.tile([128, 1], F32, tag="tw")
            nc.scalar.activation(tw[:], epst[:], AF.Ln, bias=epst[:])
            nc.scalar.activation(tw[:], tw[:], AF.Exp)

            # pos embeddings / gumbel, pair-stacked: [128, 2, 64] with
            # partitions 0:64 = bh {0, 2} (even in pair), 64:128 = bh {1, 3}.
            def load_stacked(dst, src_handle):
                v = src_handle[:].rearrange("(p v) r d -> v r p d", p=2, v=2)
                nc.sync.dma_start(dst[0:64, :, :], v[0])
                nc.sync.dma_start(dst[64:128, :, :], v[1])

            posq = persistp.tile([128, PAIRS, DIM], F32, tag="posq")
            load_stacked(posq, qp_d)
            posk = persistp.tile([128, PAIRS, DIM], F32, tag="posk")
            load_stacked(posk, kp_d)
            gum = persistp.tile([128, PAIRS, BUCKETS], F32, tag="gum")
            load_stacked(gum, g_d)

            # block-diagonal K tiles, zeroed once so off-diagonal quadrants
            # stay 0 for the packed matvecs
            Aps = []
            for pi in range(PAIRS):
                Ap = persistp.tile([128, 128], F32, tag=f"Ap{pi}")
                nc.vector.memset(Ap[:], 0.0)
                Aps.append(Ap)

            seed = persistp.tile([128, PAIRS], F32, tag="seed")  # K row sums

            def pe_bucket_sums(src, pi, pos, tag):
                # psum-accumulating fp32r matmuls; PE streams the data into
                # PSUM, DVE finishes the 8-way reduce and adds pos
                view = src[2 * pi : 2 * pi + 2].rearrange(
                    "b (bu c rl) d -> (b bu) c (rl d)", bu=BUCKETS, c=4, rl=32
                )
                acc = pacc.tile([128, DIM, 8], F32, tag="acc")
                for c in range(4):
                    chunk = datap.tile([128, 32 * DIM], F32R, tag="dq")
                    nc.gpsimd.dma_start(chunk[:], view[:, c])
                    dv = chunk[:].rearrange(
                        "p (ro ri d) -> p ro d ri", ro=4, ri=8, d=DIM
                    )
                    for j in range(4):
                        nc.tensor.matmul(
                            acc[:],
                            identr[:],
                            dv[:, j],
                            start=(c == 0 and j == 0),
                            stop=(c == 3 and j == 3),
                        )
                s_sb = workp.tile([128, DIM], F32, tag="s")
                nc.vector.reduce_sum(s_sb[:], acc[:], axis=AX.X)
                nc.vector.tensor_add(s_sb[:], s_sb[:], pos[:, pi, :])
                tps = ptr.tile([64, 128], F32, tag="tp")
                nc.tensor.transpose(tps[:], s_sb[:], ident[:])
                t_sb = persistp.tile([64, 128], F32, tag=tag)
                nc.vector.tensor_copy(t_sb[:], tps[:])
                return t_sb

            def dve_bucket_sums(src, pi, pos, tag):
                # DVE strided chunk reduces; mixed chunk sizes so the last
                # chunk's reduce (tail critical path) is small
                rls = (16, 16, 16, 16, 16, 16, 8, 8, 8, 8)
                ks = workp.tile([128, DIM], F32, tag=f"ks{pi}", bufs=1)
                nc.vector.tensor_copy(ks[:], pos[:, pi, :])
                row = 0
                for rl in rls:
                    kview = src[2 * pi : 2 * pi + 2].rearrange(
                        "b (bu r) d -> (b bu) r d", bu=BUCKETS, r=128
                    )[:, row : row + rl]
                    row += rl
                    kchunk = datap.tile(
                        [128, rl * DIM], F32, tag=f"dk{rl}", bufs=4
                    )
                    nc.gpsimd.dma_start(
                        kchunk[:].rearrange("p (r d) -> p r d", r=rl, d=DIM), kview
                    )
                    kred = workp.tile([128, DIM], F32, tag="kred")
                    nc.vector.reduce_sum(
                        kred[:],
                        kchunk[:].rearrange("p (rr d) -> p d rr", rr=rl, d=DIM),
                        axis=AX.X,
                    )
                    nc.vector.tensor_add(ks[:], ks[:], kred[:])
                tpk = ptr.tile([64, 128], F32, tag="tp")
                nc.tensor.transpose(tpk[:], ks[:], ident[:])
                t_sb = persistp.tile([64, 128], F32, tag=tag)
                nc.vector.tensor_copy(t_sb[:], tpk[:])
                return t_sb

            for pi in range(PAIRS):
                sTq = pe_bucket_sums(q_d, pi, posq, f"sTq{pi}")
                if pi == PAIRS - 1:
                    sTk = dve_bucket_sums(k_d, pi, posk, f"sTk{pi}")
                else:
                    sTk = pe_bucket_sums(k_d, pi, posk, f"sTk{pi}")

                # R[i, j] = sum_d sq[i, d] sk[j, d]; bh pair stacked on partitions
                Rps = pR.tile([128, BUCKETS], F32, tag="R")
                for v in range(2):
                    nc.tensor.matmul(
                        Rps[64 * v : 64 * (v + 1), :],
                        sTq[:, 64 * v : 64 * (v + 1)],
                        sTk[:, 64 * v : 64 * (v + 1)],
                        start=True,
                        stop=True,
                    )

                # K = exp((ln(relu(R)+eps) + g) / T) written block-diagonally,
                # with row sums accumulated as the first Sinkhorn seed
                # (b_0 = ones => a_1 = 1/rowsums).
                t1 = workp.tile([128, BUCKETS], F32, tag="t1a")
                nc.vector.tensor_scalar_max(t1[:], Rps[:], 0.0)
                t2 = workp.tile([128, BUCKETS], F32, tag="t1b")
                nc.scalar.activation(t2[:], t1[:], AF.Ln, bias=epst[:])
                nc.vector.tensor_add(t2[:], t2[:], gum[:, pi, :])
                Ap = Aps[pi]
                nc.scalar.activation(
                    Ap[0:64, 0:64], t2[0:64, :], AF.Exp,
                    scale=1.0 / TEMP,
                    accum_out=seed[0:64, pi : pi + 1],
                )
                nc.scalar.activation(
                    Ap[64:128, 64:128], t2[64:128, :], AF.Exp,
                    scale=1.0 / TEMP,
                    accum_out=seed[64:128, pi : pi + 1],
                )

                # bf16 copies of K and K^T for the matvec chain; fp32 K^T for
                # the final assembly.
                with nc.allow_low_precision("sinkhorn matvecs in bf16"):
                    Kbf = persistp.tile([128, 128], BF16, tag=f"Kbf{pi}")
                    nc.vector.tensor_copy(Kbf[:], Ap[:])
                    a_bf = chainp.tile([128, 1], BF16, tag=f"a{pi}")
                    nc.vector.reciprocal(a_bf[:], seed[:, pi : pi + 1])
                    tpA = pT.tile([128, 128], F32, tag="tpA")
                    nc.tensor.transpose(tpA[:], Ap[:], ident[:])
                    ApT = persistp.tile([128, 128], F32, tag=f"ApT{pi}")
                    nc.vector.tensor_copy(ApT[:], tpA[:])
                    KTbf = persistp.tile([128, 128], BF16, tag=f"KTbf{pi}")
                    nc.vector.tensor_copy(KTbf[:], tpA[:])

                    # Sinkhorn chain on scaling vectors:
                    #   a_t = 1/(K b_{t-1}) [seed for t=1], b_t = 1/(K^T a_t)
                    a_f32 = persistp.tile([128, 1], F32, tag=f"af{pi}")
                    b_f32 = persistp.tile([128, 1], F32, tag=f"bf{pi}")
                    for t in range(1, SINKHORN_ITER + 1):
                        # b_t = 1/(K^T a_t): contraction over i -> lhsT = K
                        mv = pmv.tile([128, 1], F32, tag="mv")
                        nc.tensor.matmul(mv[:], Kbf[:], a_bf[:], start=True, stop=True)
                        if t == SINKHORN_ITER:
                            nc.vector.reciprocal(b_f32[:], mv[:])
                        else:
                            b_bf = chainp.tile([128, 1], BF16, tag=f"b{pi}")
                            nc.vector.reciprocal(b_bf[:], mv[:])
                            # a_{t+1} = 1/(K b_t): contraction over j -> lhsT = K^T
                            mv2 = pmv.tile([128, 1], F32, tag="mv")
                            nc.tensor.matmul(mv2[:], KTbf[:], b_bf[:], start=True, stop=True)
                            a_bf = chainp.tile([128, 1], BF16, tag=f"a{pi}")
                            nc.vector.reciprocal(a_bf[:], mv2[:])
                            if t == SINKHORN_ITER - 1:
                                nc.vector.reciprocal(a_f32[:], mv2[:])

                # final E = diag(a_8) K diag(b_8) = diag(a) (diag(b) K^T)^T
                Tb = workp.tile([128, 128], F32, tag="Tb")
                nc.vector.tensor_scalar_mul(Tb[:], ApT[:], b_f32[:])
                tpF = pT.tile([128, 128], F32, tag="tpA")
                nc.tensor.transpose(tpF[:], Tb[:], ident[:])
                osb = persistp.tile([128, 128], F32, tag=f"osb{pi}")
                nc.vector.tensor_scalar_mul(osb[:], tpF[:], a_f32[:])

                # split the two quadrant DMAs across engines so descriptor
                # generation for the tail pair is parallel
                nc.sync.dma_start(out_d[2 * pi], osb[0:64, 0:64])
                nc.scalar.dma_start(out_d[2 * pi + 1], osb[64:128, 64:128])

    nc.compile()
    return nc


_NC = None


def _get_program():
    global _NC
    if _NC is None:
        _NC = _build_program()
    return _NC


def _make_in_maps(inputs):
    q = np.ascontiguousarray(inputs["q"], dtype=np.float32)
    k = np.ascontiguousarray(inputs["k"], dtype=np.float32)
    qpe = np.asarray(inputs["q_pos_emb"], dtype=np.float32)
    kpe = np.asarray(inputs["k_pos_emb"], dtype=np.float32)
    g = np.ascontiguousarray(inputs["gumbel"], dtype=np.float32)

    b = BH // HEADS
    # device computes bucket SUMS: fold the /128 mean into pos*128 and the
    # resulting /128^2 einsum scale into gumbel - ln(128^2) (exact in the
    # log domain of the sinkhorn kernel)
    qpos = (np.broadcast_to(qpe, (b, HEADS, BUCKETS, DIM)).reshape(BH, BUCKETS, DIM)
            * 128.0).astype(np.float32)
    kpos = (np.broadcast_to(kpe, (b, HEADS, BUCKETS, DIM)).reshape(BH, BUCKETS, DIM)
            * 128.0).astype(np.float32)
    gshift = (g.astype(np.float64) - np.log(SCALE)).astype(np.float32)
    ident = np.eye(128, dtype=np.float32)

    in_maps = []
    for c in range(N_CORES):
        sl = slice(NBH * c, NBH * (c + 1))
        in_maps.append(
            {
                "q": np.ascontiguousarray(q[sl]),
                "k": np.ascontiguousarray(k[sl]),
                "qpos": np.ascontiguousarray(qpos[sl]),
                "kpos": np.ascontiguousarray(kpos[sl]),
                "gumbel": np.ascontiguousarray(gshift[sl]),
                "ident": ident,
                "identr": ident,
            }
        )
    return in_maps


def run(inputs, trace=False):
    nc = _get_program()
    in_maps = _make_in_maps(inputs)
    res = run_bass_kernel_spmd(
        nc, in_maps, core_ids=list(range(N_CORES)), trace=trace
    )
    out = np.concatenate(
        [res.results[c]["out"] for c in range(N_CORES)], axis=0
    ).astype(np.float32)
    return out, res


def kernel(**inputs) -> np.ndarray:
    out, _ = run(inputs, trace=False)
    return out


# revision 10
# speedup vs baseline: 1.1427x; 1.1117x over previous
"""Trainium2 Bass kernel for nn_AttentionSortNet (sparse_attention).

Per bh slice (data-parallel over bh across 8 cores):
  b_q = bucket-mean(q), b_k = bucket-mean(k)          (64 buckets x 128 elems)
  sq = b_q + q_pos, sk = b_k + k_pos
  R  = sq @ sk^T                                       (64 x 64)
  K  = exp((ln(relu(R)+eps) + gumbel) / T)
  8x Sinkhorn row/col normalization; out = final E

Device mapping (per core, 4 bh = 2 bh-pairs):
  - q/k pair loads: 1 MiB SWDGE DMAs per (pair, tensor) into tiles
    [128, 2048] with partition = (bh-in-pair, bucket), free = (seq r, dim d).
    The gpsimd queue carries ONLY these chunk DMAs. The last stream
    (pair-1 k) uses 8 x 512 KiB chunks to shrink the post-DMA tail.
  - bucket SUMS (not means): q via accumulating fp32r matmuls on the PE
    (identity stationary); k via DVE strided reduces (PE and DVE each
    handle ~half the 16 MiB so both fit inside the DMA window).
    The /128 mean and /128^2 einsum scales are folded host-side into
    pos*128 and gumbel - ln(128^2), which is exact.
  - ACT tables: a custom insert_act_table_loads pins Ln and Exp to the
    combined natural_log_exp_and_others table, eliminating the 1.28us
    table reload that otherwise sits between Ln and Exp per pair.
  - Sinkhorn via scaling vectors instead of 16 full-matrix transposes:
      a_t = 1/(K b_{t-1}),  b_t = 1/(K^T a_t),  b_0 = 1
    K per pair is packed BLOCK-DIAGONALLY in a [128,128] tile (bh even in
    [0:64,0:64], bh odd in [64:128,64:128], zeros elsewhere) so one
    1-column bf16 matvec serves both bh. a_1 comes free from the Exp
    activation's accum_out row sums. Final E = diag(a_8) K diag(b_8) is
    assembled as diag(a) * transpose(diag(b) K^T) with fp32 K.
  - Pair 0's whole chain overlaps the DMA phase; only pair 1's chain
    trails the last chunk.
"""
import sys

sys.path.insert(0, "/opt/trn_rl_repo")

import numpy as np

import bass_rust as _bass_rust
import concourse.bass as bass
import concourse.bacc as bacc
import concourse.mybir as mybir
from concourse import tile
from concourse.bass_utils import run_bass_kernel_spmd
from concourse.hw_specs import get_activation_tables

HEADS = 8
BUCKETS = 64
DIM = 64
TEMP = 0.7
EPS = 1e-6
N_CORES = 8
BH = 32
SEQ = 8192
NBH = BH // N_CORES        # 4 bh per core
PAIRS = NBH // 2           # 2 bh-pairs per core
SINKHORN_ITER = 8
SCALE = 128.0 * 128.0      # bucket-sum (not mean) einsum scale, folded on host
EPS_S = EPS * SCALE        # matching eps for ln(relu(R_scaled) + eps_s)

F32 = mybir.dt.float32
F32R = mybir.dt.float32r
BF16 = mybir.dt.bfloat16
AF = mybir.ActivationFunctionType
AX = mybir.AxisListType
ALU = mybir.AluOpType

COMBINED_TABLE = "natural_log_exp_and_others"


class _Bacc(bacc.Bacc):
    """Bacc whose ACT-table placement serves Ln and Exp from the single
    combined table, so alternating Ln/Exp never reloads tables."""

    def insert_act_table_loads(self):
        has_activation = any(
            isinstance(i, mybir.InstActivation)
            for b in self.main_func.blocks
            for i in b.instructions
        )
        if not has_activation:
            return
        tables = []
        for name, fns in get_activation_tables(self.m.arch).items():
            if name != COMBINED_TABLE:
                fns = fns - {AF.Ln, AF.Exp}
            tables.append((name, fns))
        _bass_rust.insert_act_table_loads(self, tables)


def _build_program():
    nc = _Bacc("TRN2", target_bir_lowering=False, debug=False, num_devices=N_CORES)

    q_d = nc.dram_tensor("q", [NBH, SEQ, DIM], F32, kind="ExternalInput")
    k_d = nc.dram_tensor("k", [NBH, SEQ, DIM], F32, kind="ExternalInput")
    qp_d = nc.dram_tensor("qpos", [NBH, BUCKETS, DIM], F32, kind="ExternalInput")
    kp_d = nc.dram_tensor("kpos", [NBH, BUCKETS, DIM], F32, kind="ExternalInput")
    g_d = nc.dram_tensor("gumbel", [NBH, BUCKETS, BUCKETS], F32, kind="ExternalInput")
    id_d = nc.dram_tensor("ident", [128, 128], F32, kind="ExternalInput")
    idr_d = nc.dram_tensor("identr", [128, 128], F32R, kind="ExternalInput")
    out_d = nc.dram_tensor("out", [NBH, BUCKETS, BUCKETS], F32, kind="ExternalOutput")

    with tile.TileContext(nc) as tc:
        with (
            tc.tile_pool(name="const", bufs=1) as constp,
            tc.tile_pool(name="data", bufs=6) as datap,
            tc.tile_pool(name="work", bufs=2) as workp,
            tc.tile_pool(name="persist", bufs=1) as persistp,
            tc.tile_pool(name="chain", bufs=2) as chainp,
            tc.tile_pool(name="pacc", bufs=3, space=bass.MemorySpace.PSUM) as pacc,
            tc.tile_pool(name="ptr", bufs=1, space=bass.MemorySpace.PSUM) as ptr,
            tc.tile_pool(name="pR", bufs=1, space=bass.MemorySpace.PSUM) as pR,
            tc.tile_pool(name="pT", bufs=2, space=bass.MemorySpace.PSUM) as pT,
            tc.tile_pool(name="pmv", bufs=1, space=bass.MemorySpace.PSUM) as pmv,
        ):
            # identities via HWDGE: fp32 for transposes, fp32r (pre-rounded
            # on host; 0/1 are exact) as the q bucket-sum stationary.
            identr = constp.tile([128, 128], F32R, tag="identr")
            nc.sync.dma_start(identr[:], idr_d[:])
            ident = constp.tile([128, 128], F32, tag="ident")
            nc.sync.dma_start(ident[:], id_d[:])

            epst = constp.tile([128, 1], F32, tag="eps")
            nc.vector.memset(epst[:], EPS_S)

            # ACT warm-up: loads the combined Ln+Exp table once.
            tw = constp.tile([128, 1], F32, tag="tw")
            nc.scalar.activation(tw[:], epst[:], AF.Ln, bias=epst[:])
            nc.scalar.activation(tw[:], tw[:], AF.Exp)

            # pos embeddings / gumbel, pair-stacked: [128, 2, 64] with
            # partitions 0:64 = bh {0, 2} (even in pair), 64:128 = bh {1, 3}.
            def load_stacked(dst, src_handle):
                v = src_handle[:].rearrange("(p v) r d -> v r p d", p=2, v=2)
                nc.sync.dma_start(dst[0:64, :, :], v[0])
                nc.sync.dma_start(dst[64:128, :, :], v[1])

            posq = persistp.tile([128, PAIRS, DIM], F32, tag="posq")
            load_stacked(posq, qp_d)
            posk = persistp.tile([128, PAIRS, DIM], F32, tag="posk")
            load_stacked(posk, kp_d)
            gum = persistp.tile([128, PAIRS, BUCKETS], F32, tag="gum")
            load_stacked(gum, g_d)

            # block-diagonal K tiles, zeroed once so off-diagonal quadrants
            # stay 0 for the packed matvecs
            Aps = []
            for pi in range(PAIRS):
                Ap = persistp.tile([128, 128], F32, tag=f"Ap{pi}")
                nc.vector.memset(Ap[:], 0.0)
                Aps.append(Ap)

            seed = persistp.tile([128, PAIRS], F32, tag="seed")  # K row sums

            def pe_bucket_sums(src, pi, pos, tag, rls=(32, 32, 32, 32)):
                # psum-accumulating fp32r matmuls; PE streams the data into
                # PSUM, DVE finishes the 8-way reduce and adds pos. rls sets
                # per-chunk seq rows (smaller tail chunks for the last stream).
                nch = len(rls)
                acc = pacc.tile([128, DIM, 8], F32, tag="acc")
                row = 0
                for c, rl in enumerate(rls):
                    view = src[2 * pi : 2 * pi + 2].rearrange(
                        "b (bu r) d -> (b bu) r d", bu=BUCKETS, r=128
                    )[:, row : row + rl]
                    row += rl
                    chunk = datap.tile(
                        [128, rl * DIM], F32R, tag=f"dq{rl}", bufs=6 if rl == 32 else 2
                    )
                    nc.gpsimd.dma_start(
                        chunk[:].rearrange("p (r d) -> p r d", r=rl, d=DIM), view
                    )
                    ro = rl // 8
                    dv = chunk[:].rearrange(
                        "p (ro ri d) -> p ro d ri", ro=ro, ri=8, d=DIM
                    )
                    for j in range(ro):
                        nc.tensor.matmul(
                            acc[:],
                            identr[:],
                            dv[:, j],
                            start=(c == 0 and j == 0),
                            stop=(c == nch - 1 and j == ro - 1),
                        )
                s_sb = workp.tile([128, DIM], F32, tag="s")
                nc.vector.reduce_sum(s_sb[:], acc[:], axis=AX.X)
                nc.vector.tensor_add(s_sb[:], s_sb[:], pos[:, pi, :])
                tps = ptr.tile([64, 128], F32, tag="tp")
                nc.tensor.transpose(tps[:], s_sb[:], ident[:])
                t_sb = persistp.tile([64, 128], F32, tag=tag)
                nc.vector.tensor_copy(t_sb[:], tps[:])
                return t_sb

            for pi in range(PAIRS):
                sTq = pe_bucket_sums(q_d, pi, posq, f"sTq{pi}")
                last = pi == PAIRS - 1
                sTk = pe_bucket_sums(
                    k_d, pi, posk, f"sTk{pi}",
                    rls=(32, 32, 32, 16, 16) if last else (32, 32, 32, 32),
                )

                # R[i, j] = sum_d sq[i, d] sk[j, d]; bh pair stacked on partitions
                Rps = pR.tile([128, BUCKETS], F32, tag="R")
                for v in range(2):
                    nc.tensor.matmul(
                        Rps[64 * v : 64 * (v + 1), :],
                        sTq[:, 64 * v : 64 * (v + 1)],
                        sTk[:, 64 * v : 64 * (v + 1)],
                        start=True,
                        stop=True,
                    )

                # K = exp((ln(relu(R)+eps) + g) / T) written block-diagonally,
                # with row sums accumulated as the first Sinkhorn seed
                # (b_0 = ones => a_1 = 1/rowsums).
                t1 = workp.tile([128, BUCKETS], F32, tag="t1a")
                nc.vector.tensor_scalar_max(t1[:], Rps[:], 0.0)
                t2 = workp.tile([128, BUCKETS], F32, tag="t1b")
                nc.scalar.activation(t2[:], t1[:], AF.Ln, bias=epst[:])
                nc.vector.tensor_add(t2[:], t2[:], gum[:, pi, :])
                Ap = Aps[pi]
                nc.scalar.activation(
                    Ap[0:64, 0:64], t2[0:64, :], AF.Exp,
                    scale=1.0 / TEMP,
                    accum_out=seed[0:64, pi : pi + 1],
                )
                nc.scalar.activation(
                    Ap[64:128, 64:128], t2[64:128, :], AF.Exp,
                    scale=1.0 / TEMP,
                    accum_out=seed[64:128, pi : pi + 1],
                )

                # bf16 copies of K and K^T for the matvec chain; fp32 K^T for
                # the final assembly.
                with nc.allow_low_precision("sinkhorn matvecs in bf16"):
                    Kbf = persistp.tile([128, 128], BF16, tag=f"Kbf{pi}")
                    nc.vector.tensor_copy(Kbf[:], Ap[:])
                    a_bf = chainp.tile([128, 1], BF16, tag=f"a{pi}")
                    nc.vector.reciprocal(a_bf[:], seed[:, pi : pi + 1])
                    tpA = pT.tile([128, 128], F32, tag="tpA")
                    nc.tensor.transpose(tpA[:], Ap[:], ident[:])
                    ApT = persistp.tile([128, 128], F32, tag=f"ApT{pi}")
                    nc.vector.tensor_copy(ApT[:], tpA[:])
                    KTbf = persistp.tile([128, 128], BF16, tag=f"KTbf{pi}")
                    nc.vector.tensor_copy(KTbf[:], tpA[:])

                    # Sinkhorn chain on scaling vectors:
                    #   a_t = 1/(K b_{t-1}) [seed for t=1], b_t = 1/(K^T a_t)
                    a_f32 = persistp.tile([128, 1], F32, tag=f"af{pi}")
                    b_f32 = persistp.tile([128, 1], F32, tag=f"bf{pi}")
                    for t in range(1, SINKHORN_ITER + 1):
                        # b_t = 1/(K^T a_t): contraction over i -> lhsT = K
                        mv = pmv.tile([128, 1], F32, tag="mv")
                        nc.tensor.matmul(mv[:], Kbf[:], a_bf[:], start=True, stop=True)
                        if t == SINKHORN_ITER:
                            nc.vector.reciprocal(b_f32[:], mv[:])
                        else:
                            b_bf = chainp.tile([128, 1], BF16, tag=f"b{pi}")
                            nc.vector.reciprocal(b_bf[:], mv[:])
                            # a_{t+1} = 1/(K b_t): contraction over j -> lhsT = K^T
                            mv2 = pmv.tile([128, 1], F32, tag="mv")
                            nc.tensor.matmul(mv2[:], KTbf[:], b_bf[:], start=True, stop=True)
                            a_bf = chainp.tile([128, 1], BF16, tag=f"a{pi}")
                            nc.vector.reciprocal(a_bf[:], mv2[:])
                            if t == SINKHORN_ITER - 1:
                                nc.vector.reciprocal(a_f32[:], mv2[:])

                # final E = diag(a_8) K diag(b_8) = diag(a) (diag(b) K^T)^T
                Tb = workp.tile([128, 128], F32, tag="Tb")
                nc.vector.tensor_scalar_mul(Tb[:], ApT[:], b_f32[:])
                tpF = pT.tile([128, 128], F32, tag="tpA")
                nc.tensor.transpose(tpF[:], Tb[:], ident[:])
                osb = persistp.tile([128, 128], F32, tag=f"osb{pi}")
                nc.vector.tensor_scalar_mul(osb[:], tpF[:], a_f32[:])

                # split the two quadrant DMAs across engines so descriptor
                # generation for the tail pair is parallel
                nc.sync.dma_start(out_d[2 * pi], osb[0:64, 0:64])
                nc.scalar.dma_start(out_d[2 * pi + 1], osb[64:128, 64:128])

    nc.compile()
    return nc


_NC = None


def _get_program():
    global _NC
    if _NC is None:
        _NC = _build_program()
    return _NC


def _make_in_maps(inputs):
    q = np.ascontiguousarray(inputs["q"], dtype=np.float32)
    k = np.ascontiguousarray(inputs["k"], dtype=np.float32)
    qpe = np.asarray(inputs["q_pos_emb"], dtype=np.float32)
    kpe = np.asarray(inputs["k_pos_emb"], dtype=np.float32)
    g = np.ascontiguousarray(inputs["gumbel"], dtype=np.float32)

    b = BH // HEADS
    # device computes bucket SUMS: fold the /128 mean into pos*128 and the
    # resulting /128^2 einsum scale into gumbel - ln(128^2) (exact in the
    # log domain of the sinkhorn kernel)
    qpos = (np.broadcast_to(qpe, (b, HEADS, BUCKETS, DIM)).reshape(BH, BUCKETS, DIM)
            * 128.0).astype(np.float32)
    kpos = (np.broadcast_to(kpe, (b, HEADS, BUCKETS, DIM)).reshape(BH, BUCKETS, DIM)
            * 128.0).astype(np.float32)
    gshift = (g.astype(np.float64) - np.log(SCALE)).astype(np.float32)
    ident = np.eye(128, dtype=np.float32)

    in_maps = []
    for c in range(N_CORES):
        sl = slice(NBH * c, NBH * (c + 1))
        in_maps.append(
            {
                "q": np.ascontiguousarray(q[sl]),
                "k": np.ascontiguousarray(k[sl]),
                "qpos": np.ascontiguousarray(qpos[sl]),
                "kpos": np.ascontiguousarray(kpos[sl]),
                "gumbel": np.ascontiguousarray(gshift[sl]),
                "ident": ident,
                "identr": ident,
            }
        )
    return in_maps


def run(inputs, trace=False):
    nc = _get_program()
    in_maps = _make_in_maps(inputs)
    res = run_bass_kernel_spmd(
        nc, in_maps, core_ids=list(range(N_CORES)), trace=trace
    )
    out = np.concatenate(
        [res.results[c]["out"] for c in range(N_CORES)], axis=0
    ).astype(np.float32)
    return out, res


def kernel(**inputs) -> np.ndarray:
    out, _ = run(inputs, trace=False)
    return out


# revision 14
# speedup vs baseline: 1.1453x; 1.0023x over previous
"""Trainium2 Bass kernel for nn_AttentionSortNet (sparse_attention).

Per bh slice (data-parallel over bh across 8 cores):
  b_q = bucket-mean(q), b_k = bucket-mean(k)          (64 buckets x 128 elems)
  sq = b_q + q_pos, sk = b_k + k_pos
  R  = sq @ sk^T                                       (64 x 64)
  K  = exp((ln(relu(R)+eps) + gumbel) / T)
  8x Sinkhorn row/col normalization; out = final E

Device mapping (per core, 4 bh = 2 bh-pairs):
  - q/k pair loads: 1 MiB SWDGE DMAs per (pair, tensor) into tiles
    [128, 2048] with partition = (bh-in-pair, bucket), free = (seq r, dim d).
    The gpsimd queue carries ONLY these chunk DMAs. The last stream
    (pair-1 k) uses 8 x 512 KiB chunks to shrink the post-DMA tail.
  - bucket SUMS (not means): q via accumulating fp32r matmuls on the PE
    (identity stationary); k via DVE strided reduces (PE and DVE each
    handle ~half the 16 MiB so both fit inside the DMA window).
    The /128 mean and /128^2 einsum scales are folded host-side into
    pos*128 and gumbel - ln(128^2), which is exact.
  - ACT tables: a custom insert_act_table_loads pins Ln and Exp to the
    combined natural_log_exp_and_others table, eliminating the 1.28us
    table reload that otherwise sits between Ln and Exp per pair.
  - Sinkhorn via scaling vectors instead of 16 full-matrix transposes:
      a_t = 1/(K b_{t-1}),  b_t = 1/(K^T a_t),  b_0 = 1
    K per pair is packed BLOCK-DIAGONALLY in a [128,128] tile (bh even in
    [0:64,0:64], bh odd in [64:128,64:128], zeros elsewhere) so one
    1-column bf16 matvec serves both bh. a_1 comes free from the Exp
    activation's accum_out row sums. Final E = diag(a_8) K diag(b_8) is
    assembled as diag(a) * transpose(diag(b) K^T) with fp32 K.
  - Pair 0's whole chain overlaps the DMA phase; only pair 1's chain
    trails the last chunk.
"""
import sys

sys.path.insert(0, "/opt/trn_rl_repo")

import numpy as np

import bass_rust as _bass_rust
import concourse.bass as bass
import concourse.bacc as bacc
import concourse.mybir as mybir
from concourse import tile
from concourse.bass_utils import run_bass_kernel_spmd
from concourse.hw_specs import get_activation_tables

HEADS = 8
BUCKETS = 64
DIM = 64
TEMP = 0.7
EPS = 1e-6
N_CORES = 8
BH = 32
SEQ = 8192
NBH = BH // N_CORES        # 4 bh per core
PAIRS = NBH // 2           # 2 bh-pairs per core
SINKHORN_ITER = 8
SCALE = 128.0 * 128.0      # bucket-sum (not mean) einsum scale, folded on host
EPS_S = EPS * SCALE        # matching eps for ln(relu(R_scaled) + eps_s)

F32 = mybir.dt.float32
F32R = mybir.dt.float32r
BF16 = mybir.dt.bfloat16
AF = mybir.ActivationFunctionType
AX = mybir.AxisListType
ALU = mybir.AluOpType

COMBINED_TABLE = "natural_log_exp_and_others"


class _Bacc(bacc.Bacc):
    """Bacc whose ACT-table placement serves Ln and Exp from the single
    combined table, so alternating Ln/Exp never reloads tables."""

    def insert_act_table_loads(self):
        has_activation = any(
            isinstance(i, mybir.InstActivation)
            for b in self.main_func.blocks
            for i in b.instructions
        )
        if not has_activation:
            return
        tables = []
        for name, fns in get_activation_tables(self.m.arch).items():
            if name != COMBINED_TABLE:
                fns = fns - {AF.Ln, AF.Exp}
            tables.append((name, fns))
        _bass_rust.insert_act_table_loads(self, tables)


def _build_program():
    nc = _Bacc("TRN2", target_bir_lowering=False, debug=False, num_devices=N_CORES)

    q_d = nc.dram_tensor("q", [NBH, SEQ, DIM], F32, kind="ExternalInput")
    k_d = nc.dram_tensor("k", [NBH, SEQ, DIM], F32, kind="ExternalInput")
    qp_d = nc.dram_tensor("qpos", [NBH, BUCKETS, DIM], F32, kind="ExternalInput")
    kp_d = nc.dram_tensor("kpos", [NBH, BUCKETS, DIM], F32, kind="ExternalInput")
    g_d = nc.dram_tensor("gumbel", [NBH, BUCKETS, BUCKETS], F32, kind="ExternalInput")
    id_d = nc.dram_tensor("ident", [128, 128], F32, kind="ExternalInput")
    idr_d = nc.dram_tensor("identr", [128, 128], F32R, kind="ExternalInput")
    out_d = nc.dram_tensor("out", [NBH, BUCKETS, BUCKETS], F32, kind="ExternalOutput")

    with tile.TileContext(nc) as tc:
        with (
            tc.tile_pool(name="const", bufs=1) as constp,
            tc.tile_pool(name="data", bufs=6) as datap,
            tc.tile_pool(name="work", bufs=2) as workp,
            tc.tile_pool(name="persist", bufs=1) as persistp,
            tc.tile_pool(name="chain", bufs=2) as chainp,
            tc.tile_pool(name="pacc", bufs=2, space=bass.MemorySpace.PSUM) as pacc,
            tc.tile_pool(name="ptr", bufs=1, space=bass.MemorySpace.PSUM) as ptr,
            tc.tile_pool(name="pR", bufs=1, space=bass.MemorySpace.PSUM) as pR,
            tc.tile_pool(name="pT", bufs=2, space=bass.MemorySpace.PSUM) as pT,
            tc.tile_pool(name="pmv", bufs=1, space=bass.MemorySpace.PSUM) as pmv,
        ):
            # identities via HWDGE: fp32 for transposes, fp32r (pre-rounded
            # on host; 0/1 are exact) as the q bucket-sum stationary.
            identr = constp.tile([128, 128], F32R, tag="identr")
            nc.sync.dma_start(identr[:], idr_d[:])
            ident = constp.tile([128, 128], F32, tag="ident")
            nc.sync.dma_start(ident[:], id_d[:])

            epst = constp.tile([128, 1], F32, tag="eps")
            nc.vector.memset(epst[:], EPS_S)

            # ACT warm-up: loads the combined Ln+Exp table once.
            tw = constp.tile([128, 1], F32, tag="tw")
            nc.scalar.activation(tw[:], epst[:], AF.Ln, bias=epst[:])
            nc.scalar.activation(tw[:], tw[:], AF.Exp)

            # pos embeddings / gumbel, pair-stacked: [128, 2, 64] with
            # partitions 0:64 = bh {0, 2} (even in pair), 64:128 = bh {1, 3}.
            def load_stacked(dst, src_handle):
                v = src_handle[:].rearrange("(p v) r d -> v r p d", p=2, v=2)
                nc.sync.dma_start(dst[0:64, :, :], v[0])
                nc.sync.dma_start(dst[64:128, :, :], v[1])

            posq = persistp.tile([128, PAIRS, DIM], F32, tag="posq")
            load_stacked(posq, qp_d)
            posk = persistp.tile([128, PAIRS, DIM], F32, tag="posk")
            load_stacked(posk, kp_d)
            gum = persistp.tile([128, PAIRS, BUCKETS], F32, tag="gum")
            load_stacked(gum, g_d)

            # block-diagonal K tiles, zeroed once so off-diagonal quadrants
            # stay 0 for the packed matvecs
            Aps = []
            for pi in range(PAIRS):
                Ap = persistp.tile([128, 128], F32, tag=f"Ap{pi}")
                nc.vector.memset(Ap[:], 0.0)
                Aps.append(Ap)

            seed = persistp.tile([128, PAIRS], F32, tag="seed")  # K row sums

            def pe_bucket_sums(src, pi, pos, tag, rls=(32, 32, 32, 32)):
                # psum-accumulating fp32r matmuls; PE streams the data into
                # PSUM, DVE finishes the 8-way reduce and adds pos. rls sets
                # per-chunk seq rows (smaller tail chunks for the last stream).
                nch = len(rls)
                acc = pacc.tile([128, DIM, 8], F32, tag="acc")
                row = 0
                for c, rl in enumerate(rls):
                    view = src[2 * pi : 2 * pi + 2].rearrange(
                        "b (bu r) d -> (b bu) r d", bu=BUCKETS, r=128
                    )[:, row : row + rl]
                    row += rl
                    chunk = datap.tile(
                        [128, rl * DIM], F32R, tag=f"dq{rl}", bufs=6 if rl == 32 else 2
                    )
                    nc.gpsimd.dma_start(
                        chunk[:].rearrange("p (r d) -> p r d", r=rl, d=DIM), view
                    )
                    ro = rl // 8
                    dv = chunk[:].rearrange(
                        "p (ro ri d) -> p ro d ri", ro=ro, ri=8, d=DIM
                    )
                    for j in range(ro):
                        nc.tensor.matmul(
                            acc[:],
                            identr[:],
                            dv[:, j],
                            start=(c == 0 and j == 0),
                            stop=(c == nch - 1 and j == ro - 1),
                        )
                s_sb = workp.tile([128, DIM], F32, tag="s")
                nc.vector.reduce_sum(s_sb[:], acc[:], axis=AX.X)
                nc.vector.tensor_add(s_sb[:], s_sb[:], pos[:, pi, :])
                tps = ptr.tile([64, 128], F32, tag="tp")
                nc.tensor.transpose(tps[:], s_sb[:], ident[:])
                t_sb = persistp.tile([64, 128], F32, tag=tag)
                nc.vector.tensor_copy(t_sb[:], tps[:])
                return t_sb

            prep = []
            for pi in range(PAIRS):
                sTq = pe_bucket_sums(q_d, pi, posq, f"sTq{pi}")
                last = pi == PAIRS - 1
                sTk = pe_bucket_sums(
                    k_d, pi, posk, f"sTk{pi}",
                    rls=(32, 32, 32, 16, 16) if last else (32, 32, 32, 32),
                )

                # R[i, j] = sum_d sq[i, d] sk[j, d]; bh pair stacked on partitions
                Rps = pR.tile([128, BUCKETS], F32, tag="R")
                for v in range(2):
                    nc.tensor.matmul(
                        Rps[64 * v : 64 * (v + 1), :],
                        sTq[:, 64 * v : 64 * (v + 1)],
                        sTk[:, 64 * v : 64 * (v + 1)],
                        start=True,
                        stop=True,
                    )

                # K = exp((ln(relu(R)+eps) + g) / T) written block-diagonally,
                # with row sums accumulated as the first Sinkhorn seed
                # (b_0 = ones => a_1 = 1/rowsums).
                t1 = workp.tile([128, BUCKETS], F32, tag="t1a")
                nc.vector.tensor_scalar_max(t1[:], Rps[:], 0.0)
                t2 = workp.tile([128, BUCKETS], F32, tag="t1b")
                nc.scalar.activation(t2[:], t1[:], AF.Ln, bias=epst[:])
                nc.vector.tensor_add(t2[:], t2[:], gum[:, pi, :])
                Ap = Aps[pi]
                nc.scalar.activation(
                    Ap[0:64, 0:64], t2[0:64, :], AF.Exp,
                    scale=1.0 / TEMP,
                    accum_out=seed[0:64, pi : pi + 1],
                )
                nc.scalar.activation(
                    Ap[64:128, 64:128], t2[64:128, :], AF.Exp,
                    scale=1.0 / TEMP,
                    accum_out=seed[64:128, pi : pi + 1],
                )

                # bf16 copies of K and K^T for the matvec chain; fp32 K^T for
                # the final assembly.
                with nc.allow_low_precision("sinkhorn matvecs in bf16"):
                    Kbf = persistp.tile([128, 128], BF16, tag=f"Kbf{pi}")
                    nc.vector.tensor_copy(Kbf[:], Ap[:])
                    a_bf = chainp.tile([128, 1], BF16, tag=f"a{pi}")
                    nc.vector.reciprocal(a_bf[:], seed[:, pi : pi + 1])
                    tpA = pT.tile([128, 128], F32, tag="tpA")
                    nc.tensor.transpose(tpA[:], Ap[:], ident[:])
                    ApT = persistp.tile([128, 128], F32, tag=f"ApT{pi}")
                    nc.vector.tensor_copy(ApT[:], tpA[:])
                    KTbf = persistp.tile([128, 128], BF16, tag=f"KTbf{pi}")
                    nc.vector.tensor_copy(KTbf[:], tpA[:])
                prep.append({"Kbf": Kbf, "KTbf": KTbf, "ApT": ApT, "a": a_bf})

            # Both pairs' Sinkhorn chains interleaved AFTER all means in the
            # PE stream: the two serial chains hide each other's reciprocal
            # latency, and keeping their dependent matvecs out of the
            # DMA-paced mean window avoids head-of-line blocking there.
            #   a_t = 1/(K b_{t-1}) [seed for t=1], b_t = 1/(K^T a_t)
            a_f32 = [
                persistp.tile([128, 1], F32, tag=f"af{pi}", name=f"af{pi}")
                for pi in range(PAIRS)
            ]
            b_f32 = [
                persistp.tile([128, 1], F32, tag=f"bf{pi}", name=f"bf{pi}")
                for pi in range(PAIRS)
            ]
            with nc.allow_low_precision("sinkhorn matvecs in bf16"):
                for t in range(1, SINKHORN_ITER + 1):
                    mvs = []
                    for pi in range(PAIRS):
                        # b_t = 1/(K^T a_t): contraction over i -> lhsT = K
                        mv = pmv.tile([128, 1], F32, tag=f"mv{pi}")
                        nc.tensor.matmul(
                            mv[:], prep[pi]["Kbf"][:], prep[pi]["a"][:],
                            start=True, stop=True,
                        )
                        mvs.append(mv)
                    if t == SINKHORN_ITER:
                        for pi in range(PAIRS):
                            nc.vector.reciprocal(b_f32[pi][:], mvs[pi][:])
                        break
                    b_bfs = []
                    for pi in range(PAIRS):
                        b_bf = chainp.tile([128, 1], BF16, tag=f"b{pi}")
                        nc.vector.reciprocal(b_bf[:], mvs[pi][:])
                        b_bfs.append(b_bf)
                    mv2s = []
                    for pi in range(PAIRS):
                        # a_{t+1} = 1/(K b_t): contraction over j -> lhsT = K^T
                        mv2 = pmv.tile([128, 1], F32, tag=f"mv{pi}")
                        nc.tensor.matmul(
                            mv2[:], prep[pi]["KTbf"][:], b_bfs[pi][:],
                            start=True, stop=True,
                        )
                        mv2s.append(mv2)
                    for pi in range(PAIRS):
                        a_bf = chainp.tile([128, 1], BF16, tag=f"a{pi}")
                        nc.vector.reciprocal(a_bf[:], mv2s[pi][:])
                        prep[pi]["a"] = a_bf
                        if t == SINKHORN_ITER - 1:
                            nc.vector.reciprocal(a_f32[pi][:], mv2s[pi][:])

            # final E = diag(a_8) K diag(b_8) = diag(a) (diag(b) K^T)^T,
            # both pairs interleaved
            Tbs, tpFs = [], []
            for pi in range(PAIRS):
                Tb = workp.tile([128, 128], F32, tag=f"Tb{pi}", bufs=1)
                nc.vector.tensor_scalar_mul(Tb[:], prep[pi]["ApT"][:], b_f32[pi][:])
                Tbs.append(Tb)
            for pi in range(PAIRS):
                tpF = pT.tile([128, 128], F32, tag="tpA")
                nc.tensor.transpose(tpF[:], Tbs[pi][:], ident[:])
                tpFs.append(tpF)
            osbs = []
            for pi in range(PAIRS):
                osb = persistp.tile([128, 128], F32, tag=f"osb{pi}")
                nc.vector.tensor_scalar_mul(osb[:], tpFs[pi][:], a_f32[pi][:])
                osbs.append(osb)
            for pi in range(PAIRS):
                # quadrant DMAs split across the two HWDGE engines
                nc.sync.dma_start(out_d[2 * pi], osbs[pi][0:64, 0:64])
                nc.scalar.dma_start(out_d[2 * pi + 1], osbs[pi][64:128, 64:128])

    nc.compile()
    return nc


_NC = None


def _get_program():
    global _NC
    if _NC is None:
        _NC = _build_program()
    return _NC


def _make_in_maps(inputs):
    q = np.ascontiguousarray(inputs["q"], dtype=np.float32)
    k = np.ascontiguousarray(inputs["k"], dtype=np.float32)
    qpe = np.asarray(inputs["q_pos_emb"], dtype=np.float32)
    kpe = np.asarray(inputs["k_pos_emb"], dtype=np.float32)
    g = np.ascontiguousarray(inputs["gumbel"], dtype=np.float32)

    b = BH // HEADS
    # device computes bucket SUMS: fold the /128 mean into pos*128 and the
    # resulting /128^2 einsum scale into gumbel - ln(128^2) (exact in the
    # log domain of the sinkhorn kernel)
    qpos = (np.broadcast_to(qpe, (b, HEADS, BUCKETS, DIM)).reshape(BH, BUCKETS, DIM)
            * 128.0).astype(np.float32)
    kpos = (np.broadcast_to(kpe, (b, HEADS, BUCKETS, DIM)).reshape(BH, BUCKETS, DIM)
            * 128.0).astype(np.float32)
    gshift = (g.astype(np.float64) - np.log(SCALE)).astype(np.float32)
    ident = np.eye(128, dtype=np.float32)

    in_maps = []
    for c in range(N_CORES):
        sl = slice(NBH * c, NBH * (c + 1))
        in_maps.append(
            {
                "q": np.ascontiguousarray(q[sl]),
                "k": np.ascontiguousarray(k[sl]),
                "qpos": np.ascontiguousarray(qpos[sl]),
                "kpos": np.ascontiguousarray(kpos[sl]),
                "gumbel": np.ascontiguousarray(gshift[sl]),
                "ident": ident,
                "identr": ident,
            }
        )
    return in_maps


def run(inputs, trace=False):
    nc = _get_program()
    in_maps = _make_in_maps(inputs)
    res = run_bass_kernel_spmd(
        nc, in_maps, core_ids=list(range(N_CORES)), trace=trace
    )
    out = np.concatenate(
        [res.results[c]["out"] for c in range(N_CORES)], axis=0
    ).astype(np.float32)
    return out, res


def kernel(**inputs) -> np.ndarray:
    out, _ = run(inputs, trace=False)
    return out
